# revision 3
# baseline (speedup 1.0000x reference)
"""Trainium2 Bass kernel for suffix-softmax attention visualization.

Computes, for hidden_states [S, B, H], W [H, 1], b [1]:
    s[t, b]   = sum_h hidden_states[t, b, h] * W[h, 0] + b[0]
    out[t, b] = exp(s[t, b]) / sum_{t' >= t} exp(s[t', b])     (suffix softmax)
returned as [S, B, 1] f32.

The softmax ratio is shift-invariant, so the scalar bias b cancels exactly
and is not needed on device. The scores are N(0, 1)-scaled by construction
(W drawn as randn/sqrt(H)), so exp() needs no max-subtraction in f32.

Sharding: data-parallel over the batch axis — 8 NeuronCores, 8 batch
columns each. Per core:
  - 32 blocks of [128 s, 8 b, 512 h] are streamed from HBM (2 MiB DMAs,
    16 KiB contiguous rows);
  - DVE tensor_tensor_reduce fuses the W-multiply and the h-reduction in a
    single 1x pass per (block, b);
  - ACT computes exp;
  - the per-column suffix sum uses a lower-triangular matmul on the PE for
    the within-block scan, Hillis-Steele shifted adds for the 32 block
    totals, and a K=1 ones-matmul to broadcast the cross-block offsets
    back across partitions;
  - DVE divides (reciprocal + multiply) and the result is DMA'd out as a
    dense [4096, 8] per-core tensor, reassembled on the host.
"""

import numpy as np

import concourse.bacc as bacc
import concourse.mybir as mybir
import concourse.tile as tile
from concourse import bass_utils

P = 128
S = 4096
B = 64
H = 512
N_CORES = 8
BC = B // N_CORES  # batch columns per core
NBLK = S // P
C = NBLK * BC  # score-tile columns, c = blk*BC + b


def build_program(hs_bufs=3):
    nc = bacc.Bacc("TRN2", target_bir_lowering=False, debug=False)
    hs = nc.dram_tensor("hs", [S, BC, H], mybir.dt.float32, kind="ExternalInput")
    wb = nc.dram_tensor("wb", [P, H], mybir.dt.float32, kind="ExternalInput")
    tri = nc.dram_tensor("tri", [P, P], mybir.dt.float32, kind="ExternalInput")
    onesr = nc.dram_tensor("onesr", [1, P], mybir.dt.float32, kind="ExternalInput")
    out = nc.dram_tensor("out", [S, BC], mybir.dt.float32, kind="ExternalOutput")

    with tile.TileContext(nc) as tc:
        with (
            tc.tile_pool(name="consts", bufs=1) as consts,
            tc.tile_pool(name="hsp", bufs=hs_bufs) as hsp,
            tc.tile_pool(name="work", bufs=1) as work,
            tc.tile_pool(name="psum", bufs=1, space="PSUM") as psum,
        ):
            wb_t = consts.tile([P, H], mybir.dt.float32)
            nc.sync.dma_start(out=wb_t, in_=wb.ap())
            tri_t = consts.tile([P, P], mybir.dt.float32)
            nc.sync.dma_start(out=tri_t, in_=tri.ap())
            ones_t = consts.tile([1, P], mybir.dt.float32)
            nc.sync.dma_start(out=ones_t, in_=onesr.ap())

            s_col = work.tile([P, C], mybir.dt.float32)
            dummy = work.tile([P, 1], mybir.dt.float32)

            hs_ap = hs.ap()
            for blk in range(NBLK):
                hst = hsp.tile([P, BC, H], mybir.dt.float32)
                nc.sync.dma_start(out=hst, in_=hs_ap[blk * P : (blk + 1) * P, :, :])
                for b in range(BC):
                    c = blk * BC + b
                    nc.vector.scalar_tensor_tensor(
                        out=dummy.broadcast_to((P, H)),
                        in0=hst[:, b, :],
                        scalar=1.0,
                        in1=wb_t,
                        op0=mybir.AluOpType.mult,
                        op1=mybir.AluOpType.mult,
                        accum_out=s_col[:, c : c + 1],
                    )

            e_t = work.tile([P, C], mybir.dt.float32)
            nc.scalar.activation(e_t, s_col, mybir.ActivationFunctionType.Exp)

            # Within-block inclusive suffix sums: scan_ps[m, c] = sum_{k>=m} e[k, c]
            scan_ps = psum.tile([P, C], mybir.dt.float32)
            nc.tensor.matmul(scan_ps, tri_t, e_t, start=True, stop=True)

            # Cross-block exclusive suffix offsets from block totals (row 0).
            TLEN = (NBLK + 1) * BC
            PAD = 16 * BC
            t0 = work.tile([1, TLEN + PAD], mybir.dt.float32)
            t1 = work.tile([1, TLEN + PAD], mybir.dt.float32)
            nc.vector.memset(t0, 0.0)
            nc.vector.memset(t1, 0.0)
            nc.vector.tensor_copy(t0[0:1, 0:C], scan_ps[0:1, 0:C])
            src, dst = t0, t1
            d = 1
            while d < NBLK:
                nc.vector.tensor_add(
                    dst[0:1, 0:TLEN],
                    src[0:1, 0:TLEN],
                    src[0:1, d * BC : d * BC + TLEN],
                )
                src, dst = dst, src
                d *= 2
            offsets = src[0:1, BC : BC + C]

            # Broadcast offsets across partitions: bc_ps[m, c] = offsets[c]
            bc_ps = psum.tile([P, C], mybir.dt.float32)
            nc.tensor.matmul(bc_ps, ones_t, offsets, start=True, stop=True)

            bsb = work.tile([P, C], mybir.dt.float32)
            nc.scalar.copy(bsb, bc_ps)
            ssum = work.tile([P, C], mybir.dt.float32)
            nc.vector.tensor_add(ssum, bsb, scan_ps)
            rec = work.tile([P, C], mybir.dt.float32)
            nc.vector.reciprocal(rec, ssum)
            sel = work.tile([P, C], mybir.dt.float32)
            nc.vector.tensor_mul(sel, e_t, rec)

            out_ap = out.ap().rearrange("(blk p) b -> p blk b", p=P)
            sel_ap = sel[:, :].rearrange("p (blk b) -> p blk b", b=BC)
            nc.sync.dma_start(out=out_ap, in_=sel_ap)

    nc.compile()
    return nc


_PROGRAM = None


def _get_program():
    global _PROGRAM
    if _PROGRAM is None:
        _PROGRAM = build_program()
    return _PROGRAM


def make_in_maps(hidden_states, W):
    hidden_states = np.asarray(hidden_states, dtype=np.float32)
    W = np.asarray(W, dtype=np.float32)
    wb = np.ascontiguousarray(np.broadcast_to(W[:, 0][None, :], (P, H)))
    tri = np.tril(np.ones((P, P), dtype=np.float32))
    onesr = np.ones((1, P), dtype=np.float32)
    in_maps = []
    for c in range(N_CORES):
        hs_c = np.ascontiguousarray(hidden_states[:, c * BC : (c + 1) * BC, :])
        in_maps.append({"hs": hs_c, "wb": wb, "tri": tri, "onesr": onesr})
    return in_maps


def assemble_output(results):
    cols = [results[c]["out"] for c in range(N_CORES)]
    return np.concatenate(cols, axis=1)[..., None].astype(np.float32)


def kernel(hidden_states, W, b):
    nc = _get_program()
    in_maps = make_in_maps(hidden_states, W)
    res = bass_utils.run_bass_kernel_spmd(nc, in_maps, core_ids=list(range(N_CORES)))
    return assemble_output(res.results)
